# revision 18
# baseline (speedup 1.0000x reference)
"""ChordMixerBlock Trainium2 kernel.

Math (per batch b):
    h   = gelu(data @ w1 + b1)            # exact gelu
    y   = h @ w2 + b2
    out[l, :] = rotate_chord(y)[l, :] + data[l, :]
where rotate_chord rolls track t (channels [16t, 16t+16)) forward by
s_t = 2^(t-1) positions along L (track 0: no shift; track 15: 2^14 == L
-> no shift).

Sharding: 8 cores = (batch b, L-half j); each core computes y for its own
8192-token chunk in transposed layout [256 d, 8192 l] so the contraction
dim D lands on SBUF partitions (host pre-transposes inputs and transposes
the output back).

Roll handling: entirely in data layout, zero on-device communication.
  * acc[c, p] = y[c, p] + b2[c] + dataS[c, p], where dataS is the residual
    pre-rolled by +s_t per track on the HOST (pure sharding-layout prep):
    dataS[c, p] = data[(chunk0 + p - s_t) mod L, c].  Then acc[c, p] is
    exactly out[(chunk0 + p - s_t) mod L, c] for track t's channels.
  * The device stores acc contiguously (in bf16); the HOST applies the
    per-track circular roll while unsharding (a pure index permutation of
    the gathered result, the mirror image of the input-side pre-roll).
"""

import sys

sys.path.insert(0, "/opt/trn_rl_repo")

import numpy as np
import ml_dtypes

import concourse.bass as bass
import concourse.bacc as bacc
import concourse.tile as tile
import concourse.mybir as mybir
from concourse import bass_utils

B, L, D, H = 4, 16384, 256, 512
N_CORES = 8
LC = L // 2                      # per-core chunk length
NT, TS = 16, 16                  # tracks, track size
SHIFTS = [0] + [2 ** i for i in range(NT - 1)]
SEFF = [s % L for s in SHIFTS]   # track 15 -> 0
TILE = 512                       # l-tile width for matmuls
NTILES = LC // TILE
BLK = 2048                       # DMA block width
NBLK = LC // BLK

F32 = mybir.dt.float32
BF16 = mybir.dt.bfloat16


def _build():
    nc = bacc.Bacc("TRN2", target_bir_lowering=False, debug=False, num_devices=1)

    dataM_h = nc.dram_tensor("dataM", [D, LC], BF16, kind="ExternalInput")
    dataS_h = nc.dram_tensor("dataS", [D, LC], BF16, kind="ExternalInput")
    w1_h = nc.dram_tensor("w1b", [D, H], BF16, kind="ExternalInput")
    w2_h = nc.dram_tensor("w2b", [H, D], BF16, kind="ExternalInput")
    b1_h = nc.dram_tensor("b1m", [128, H // 128], F32, kind="ExternalInput")
    b2_h = nc.dram_tensor("b2m", [128, D // 128], F32, kind="ExternalInput")
    outT_h = nc.dram_tensor("outT", [D, LC], BF16, kind="ExternalOutput")

    with tile.TileContext(nc) as tc:
        with (
            tc.tile_pool(name="const", bufs=1) as cpool,
            tc.tile_pool(name="big", bufs=1) as big,
            tc.tile_pool(name="hbf", bufs=8) as hbfp,
            tc.tile_pool(name="ph", bufs=6, space="PSUM") as php,
            tc.tile_pool(name="py", bufs=2, space="PSUM") as pyp,
        ):
            # --- persistent chunk buffers (declared early so the first dm
            # pieces can lead the HWDGE rings) ---
            dm = [big.tile([128, LC], BF16, tag=f"dm{k}", name=f"dm{k}")
                  for k in range(2)]

            # first tile's dm columns lead the (uncontended) HWDGE rings:
            # lowest-latency path to the very first matmul
            nc.sync.dma_start(dm[0][:, 0:TILE], dataM_h.ap()[0:128, 0:TILE])
            nc.scalar.dma_start(dm[1][:, 0:TILE], dataM_h.ap()[128:256, 0:TILE])

            # --- weights / biases on the HWDGE rings; w1's first 128
            # columns ship as their own piece so the first LDWEIGHTS isn't
            # gated on the full tile ---
            w1sb = []
            for dt in range(2):
                w = cpool.tile([128, H], BF16, tag=f"w1_{dt}", name=f"w1sb{dt}")
                eng = nc.sync if dt == 0 else nc.scalar
                rows = slice(dt * 128, (dt + 1) * 128)
                eng.dma_start(w[:, 0:128], w1_h.ap()[rows, 0:128])
                eng.dma_start(w[:, 128:H], w1_h.ap()[rows, 128:H])
                w1sb.append(w)
            b1sb = cpool.tile([128, H // 128], F32, tag="b1")
            nc.sync.dma_start(b1sb[:], b1_h.ap())
            b2sb = cpool.tile([128, D // 128], F32, tag="b2")
            nc.scalar.dma_start(b2sb[:], b2_h.ap())
            w2sb = [cpool.tile([128, D], BF16, tag=f"w2_{ht}", name=f"w2sb{ht}")
                    for ht in range(4)]
            for ht in range(4):
                eng = nc.sync if ht % 2 == 0 else nc.scalar
                eng.dma_start(w2sb[ht][:], w2_h.ap()[ht * 128:(ht + 1) * 128, :])

            # --- remaining persistent chunk buffers ---
            rst = [big.tile([128, LC], BF16, tag=f"rst{k}", name=f"rst{k}")
                   for k in range(2)]
            acc = [big.tile([128, LC], BF16, tag=f"acc{k}", name=f"acc{k}")
                   for k in range(2)]

            # All dm/rst input streaming goes on the SWDGE (Pool) queue:
            # each SWDGE transfer is spread across all 16 SDMA engines
            # (~1-2us for 512KB), whereas a HWDGE direct2D transfer lands on
            # ~one engine (~7-16us for 512KB).  Q7 dispatch is ~0.63us per
            # DMA and the queue never stalls (loads have no input deps), so
            # queue order == priority order.  The HWDGE rings carry only
            # weights and output stores.
            def load_dm(lo, hi):
                sl = slice(lo, hi)
                nc.gpsimd.dma_start(dm[0][:, sl], dataM_h.ap()[0:128, sl])
                nc.gpsimd.dma_start(dm[1][:, sl], dataM_h.ap()[128:256, sl])

            def load_rst(blk):
                sl = slice(blk * BLK, (blk + 1) * BLK)
                for k in range(2):
                    rows = slice(k * 128, (k + 1) * 128)
                    nc.gpsimd.dma_start(rst[k][:, sl], dataS_h.ap()[rows, sl])

            load_dm(TILE, BLK)        # cols [0:TILE] went via HWDGE above
            load_rst(0)
            load_dm(BLK, 2 * BLK)
            load_rst(1)
            for blk in range(2, NBLK):
                load_dm(blk * BLK, (blk + 1) * BLK)
                load_rst(blk)

            def emit_fc1(i):
                csl = slice(i * TILE, (i + 1) * TILE)
                hbf = []
                for ht in range(4):
                    ph = php.tile([128, TILE], F32, tag="ph",
                                  name=f"ph_{i}_{ht}")
                    hs = slice(ht * 128, (ht + 1) * 128)
                    nc.tensor.matmul(
                        ph[:], w1sb[0][:, hs], dm[0][:, csl],
                        start=True, stop=False,
                    )
                    nc.tensor.matmul(
                        ph[:], w1sb[1][:, hs], dm[1][:, csl],
                        start=False, stop=True,
                    )
                    hb = hbfp.tile([128, TILE], BF16, tag="hbf",
                                   name=f"hbf_{i}_{ht}")
                    nc.scalar.activation(
                        hb[:], ph[:], mybir.ActivationFunctionType.Gelu,
                        bias=b1sb[:, ht:ht + 1],
                    )
                    hbf.append(hb)
                return hbf

            def emit_fc2(i, hbf):
                csl = slice(i * TILE, (i + 1) * TILE)
                for k in range(2):
                    py = pyp.tile([128, TILE], F32, tag="py",
                                  name=f"py_{i}_{k}")
                    ds = slice(k * 128, (k + 1) * 128)
                    for ht in range(4):
                        nc.tensor.matmul(
                            py[:], w2sb[ht][:, ds], hbf[ht][:],
                            start=(ht == 0), stop=(ht == 3),
                        )
                    # acc = (y + b2) + pre-rolled residual, rounded to bf16
                    nc.vector.scalar_tensor_tensor(
                        acc[k][:, csl], py[:], b2sb[:, k:k + 1],
                        rst[k][:, csl],
                        mybir.AluOpType.add, mybir.AluOpType.add,
                    )
                # contiguous output store per finished 1024-col slab; the
                # final slab goes out in 512-col pieces (a HWDGE transfer
                # runs on ~one SDMA engine, so smaller pieces = lower tail
                # latency after the last matmul)
                if (i + 1) % 2 == 0:
                    if i < NTILES - 1:
                        bsl = slice((i - 1) * TILE, (i + 1) * TILE)
                        nc.sync.dma_start(outT_h.ap()[0:128, bsl], acc[0][:, bsl])
                        nc.scalar.dma_start(outT_h.ap()[128:256, bsl],
                                            acc[1][:, bsl])
                    else:
                        for lo in range((i - 1) * TILE, (i + 1) * TILE, 256):
                            psl = slice(lo, lo + 256)
                            nc.sync.dma_start(outT_h.ap()[0:128, psl],
                                              acc[0][:, psl])
                            nc.scalar.dma_start(outT_h.ap()[128:256, psl],
                                                acc[1][:, psl])

            # --- software-pipelined main loop: fc1(i+1) ahead of fc2(i) ---
            prev = None
            for i in range(NTILES + 1):
                cur = emit_fc1(i) if i < NTILES else None
                if prev is not None:
                    emit_fc2(i - 1, prev)
                prev = cur

    nc.compile()
    return nc


_NC = None


def _get_nc():
    global _NC
    if _NC is None:
        _NC = _build()
    return _NC


def make_in_maps(data, w1, b1, w2, b2):
    data = np.asarray(data, dtype=np.float32)
    w1b = np.asarray(w1, dtype=np.float32).astype(ml_dtypes.bfloat16)
    w2b = np.asarray(w2, dtype=np.float32).astype(ml_dtypes.bfloat16)
    b1m = np.ascontiguousarray(
        np.asarray(b1, dtype=np.float32).reshape(H // 128, 128).T
    )
    b2m = np.ascontiguousarray(
        np.asarray(b2, dtype=np.float32).reshape(D // 128, 128).T
    )

    in_maps = []
    for bb in range(B):
        # residual pre-rolled by +s_t per track:
        # rolled[l, c] = data[(l - s_t) mod L, c]
        rolled = np.empty((L, D), dtype=np.float32)
        for t in range(NT):
            cs = slice(t * TS, (t + 1) * TS)
            rolled[:, cs] = np.roll(data[bb, :, cs], SEFF[t], axis=0)
        for j in range(2):
            sl = slice(j * LC, (j + 1) * LC)
            dataM = np.ascontiguousarray(
                data[bb, sl, :].T.astype(ml_dtypes.bfloat16)
            )
            dataS = np.ascontiguousarray(
                rolled[sl, :].T.astype(ml_dtypes.bfloat16)
            )
            in_maps.append({
                "dataM": dataM, "dataS": dataS,
                "w1b": w1b, "w2b": w2b, "b1m": b1m, "b2m": b2m,
            })
    return in_maps


def kernel(data, w1, b1, w2, b2):
    nc = _get_nc()
    in_maps = make_in_maps(data, w1, b1, w2, b2)
    res = bass_utils.run_bass_kernel_spmd(
        nc, in_maps, core_ids=list(range(N_CORES))
    )
    out = np.empty((B, L, D), dtype=np.float32)
    for bb in range(B):
        # accT[l, c] = out[(l - s_t) mod L, c]; undo with a -s_t roll
        accT = np.concatenate(
            [res.results[2 * bb + j]["outT"].T for j in range(2)], axis=0
        ).astype(np.float32)
        for t in range(NT):
            cs = slice(t * TS, (t + 1) * TS)
            out[bb, :, cs] = np.roll(accT[:, cs], -SEFF[t], axis=0)
    return out


# revision 22
# speedup vs baseline: 1.0059x; 1.0059x over previous
"""ChordMixerBlock Trainium2 kernel.

Math (per batch b):
    h   = gelu(data @ w1 + b1)            # exact gelu
    y   = h @ w2 + b2
    out[l, :] = rotate_chord(y)[l, :] + data[l, :]
where rotate_chord rolls track t (channels [16t, 16t+16)) forward by
s_t = 2^(t-1) positions along L (track 0: no shift; track 15: 2^14 == L
-> no shift).

Sharding: 8 cores = (batch b, L-half j); each core computes y for its own
8192-token chunk in transposed layout [256 d, 8192 l] so the contraction
dim D lands on SBUF partitions (host pre-transposes inputs and transposes
the output back).

Roll handling: entirely in data layout, zero on-device communication.
  * acc[c, p] = y[c, p] + b2[c] + dataS[c, p], where dataS is the residual
    pre-rolled by +s_t per track on the HOST (pure sharding-layout prep):
    dataS[c, p] = data[(chunk0 + p - s_t) mod L, c].  Then acc[c, p] is
    exactly out[(chunk0 + p - s_t) mod L, c] for track t's channels.
  * The device stores acc contiguously (in bf16); the HOST applies the
    per-track circular roll while unsharding (a pure index permutation of
    the gathered result, the mirror image of the input-side pre-roll).
"""

import sys

sys.path.insert(0, "/opt/trn_rl_repo")

import numpy as np
import ml_dtypes

import concourse.bass as bass
import concourse.bacc as bacc
import concourse.tile as tile
import concourse.mybir as mybir
from concourse import bass_utils

B, L, D, H = 4, 16384, 256, 512
N_CORES = 8
LC = L // 2                      # per-core chunk length
NT, TS = 16, 16                  # tracks, track size
SHIFTS = [0] + [2 ** i for i in range(NT - 1)]
SEFF = [s % L for s in SHIFTS]   # track 15 -> 0
TILE = 512                       # l-tile width for matmuls
NTILES = LC // TILE
BLK = 2048                       # DMA block width
NBLK = LC // BLK

F32 = mybir.dt.float32
BF16 = mybir.dt.bfloat16


def _build():
    nc = bacc.Bacc("TRN2", target_bir_lowering=False, debug=False, num_devices=1)

    dataM_h = nc.dram_tensor("dataM", [D, LC], BF16, kind="ExternalInput")
    dataS_h = nc.dram_tensor("dataS", [D, LC], BF16, kind="ExternalInput")
    w1_h = nc.dram_tensor("w1b", [D, H], BF16, kind="ExternalInput")
    w2_h = nc.dram_tensor("w2b", [H, D], BF16, kind="ExternalInput")
    b1_h = nc.dram_tensor("b1m", [128, H // 128], F32, kind="ExternalInput")
    b2_h = nc.dram_tensor("b2m", [128, D // 128], F32, kind="ExternalInput")
    outT_h = nc.dram_tensor("outT", [D, LC], BF16, kind="ExternalOutput")

    with tile.TileContext(nc) as tc:
        with (
            tc.tile_pool(name="const", bufs=1) as cpool,
            tc.tile_pool(name="big", bufs=1) as big,
            tc.tile_pool(name="hbf", bufs=8) as hbfp,
            tc.tile_pool(name="ph", bufs=6, space="PSUM") as php,
            tc.tile_pool(name="py", bufs=2, space="PSUM") as pyp,
        ):
            # --- persistent chunk buffers ---
            dm = [big.tile([128, LC], BF16, tag=f"dm{k}", name=f"dm{k}")
                  for k in range(2)]

            # --- weights / biases on the HWDGE rings; w1's first 128
            # columns ship as their own piece so the first LDWEIGHTS isn't
            # gated on the full tile ---
            w1sb = []
            for dt in range(2):
                w = cpool.tile([128, H], BF16, tag=f"w1_{dt}", name=f"w1sb{dt}")
                eng = nc.sync if dt == 0 else nc.scalar
                rows = slice(dt * 128, (dt + 1) * 128)
                eng.dma_start(w[:, 0:128], w1_h.ap()[rows, 0:128])
                eng.dma_start(w[:, 128:H], w1_h.ap()[rows, 128:H])
                w1sb.append(w)
            b1sb = cpool.tile([128, H // 128], F32, tag="b1")
            nc.sync.dma_start(b1sb[:], b1_h.ap())
            b2sb = cpool.tile([128, D // 128], F32, tag="b2")
            nc.scalar.dma_start(b2sb[:], b2_h.ap())
            w2sb = [cpool.tile([128, D], BF16, tag=f"w2_{ht}", name=f"w2sb{ht}")
                    for ht in range(4)]
            for ht in range(4):
                eng = nc.sync if ht % 2 == 0 else nc.scalar
                eng.dma_start(w2sb[ht][:], w2_h.ap()[ht * 128:(ht + 1) * 128, :])

            # --- remaining persistent chunk buffers ---
            rst = [big.tile([128, LC], BF16, tag=f"rst{k}", name=f"rst{k}")
                   for k in range(2)]
            acc = [big.tile([128, LC], BF16, tag=f"acc{k}", name=f"acc{k}")
                   for k in range(2)]

            # All dm/rst input streaming goes on the SWDGE (Pool) queue:
            # each SWDGE transfer is spread across all 16 SDMA engines
            # (~1-2us for 512KB), whereas a HWDGE direct2D transfer lands on
            # ~one engine (~7-16us for 512KB).  Q7 dispatch is ~0.63us per
            # DMA and the queue never stalls (loads have no input deps), so
            # queue order == priority order.  The HWDGE rings carry only
            # weights and output stores.
            def load_dm(lo, hi):
                sl = slice(lo, hi)
                nc.gpsimd.dma_start(dm[0][:, sl], dataM_h.ap()[0:128, sl])
                nc.gpsimd.dma_start(dm[1][:, sl], dataM_h.ap()[128:256, sl])

            def load_rst(blk):
                sl = slice(blk * BLK, (blk + 1) * BLK)
                for k in range(2):
                    rows = slice(k * 128, (k + 1) * 128)
                    nc.gpsimd.dma_start(rst[k][:, sl], dataS_h.ap()[rows, sl])

            load_dm(0, TILE)          # first tile's columns: smallest latency
            load_dm(TILE, BLK)
            load_rst(0)
            load_dm(BLK, 2 * BLK)
            load_rst(1)
            for blk in range(2, NBLK):
                load_dm(blk * BLK, (blk + 1) * BLK)
                load_rst(blk)

            def emit_fc1(i):
                csl = slice(i * TILE, (i + 1) * TILE)
                hbf = []
                for ht in range(4):
                    ph = php.tile([128, TILE], F32, tag="ph",
                                  name=f"ph_{i}_{ht}")
                    hs = slice(ht * 128, (ht + 1) * 128)
                    nc.tensor.matmul(
                        ph[:], w1sb[0][:, hs], dm[0][:, csl],
                        start=True, stop=False,
                    )
                    nc.tensor.matmul(
                        ph[:], w1sb[1][:, hs], dm[1][:, csl],
                        start=False, stop=True,
                    )
                    hb = hbfp.tile([128, TILE], BF16, tag="hbf",
                                   name=f"hbf_{i}_{ht}")
                    nc.scalar.activation(
                        hb[:], ph[:], mybir.ActivationFunctionType.Gelu,
                        bias=b1sb[:, ht:ht + 1],
                    )
                    hbf.append(hb)
                return hbf

            def emit_fc2(i, hbf):
                last = i == NTILES - 1
                # on the last tile, work in two 256-col half-tiles so the
                # closing DVE-op + store chain is as short as possible
                pieces = ([(i * TILE, i * TILE + 256),
                           (i * TILE + 256, (i + 1) * TILE)]
                          if last else [(i * TILE, (i + 1) * TILE)])
                for lo, hi in pieces:
                    csl = slice(lo, hi)
                    n = hi - lo
                    po = lo - i * TILE
                    for k in range(2):
                        py = pyp.tile([128, n], F32, tag="py",
                                      name=f"py_{i}_{k}_{po}")
                        ds = slice(k * 128, (k + 1) * 128)
                        for ht in range(4):
                            nc.tensor.matmul(
                                py[:], w2sb[ht][:, ds],
                                hbf[ht][:, po:po + n],
                                start=(ht == 0), stop=(ht == 3),
                            )
                        # acc = (y + b2) + pre-rolled residual -> bf16
                        nc.vector.scalar_tensor_tensor(
                            acc[k][:, csl], py[:],
                            b2sb[:, k:k + 1], rst[k][:, csl],
                            mybir.AluOpType.add, mybir.AluOpType.add,
                        )
                        if last:
                            eng = nc.sync if k == 0 else nc.scalar
                            eng.dma_start(
                                outT_h.ap()[k * 128:(k + 1) * 128, csl],
                                acc[k][:, csl])
                # contiguous output store per finished 1024-col slab
                if (i + 1) % 2 == 0 and not last:
                    bsl = slice((i - 1) * TILE, (i + 1) * TILE)
                    nc.sync.dma_start(outT_h.ap()[0:128, bsl], acc[0][:, bsl])
                    nc.scalar.dma_start(outT_h.ap()[128:256, bsl],
                                        acc[1][:, bsl])
                elif last:
                    bsl = slice((i - 1) * TILE, i * TILE)
                    nc.sync.dma_start(outT_h.ap()[0:128, bsl], acc[0][:, bsl])
                    nc.scalar.dma_start(outT_h.ap()[128:256, bsl],
                                        acc[1][:, bsl])

            # --- software-pipelined main loop: fc1(i+1) ahead of fc2(i) ---
            prev = None
            for i in range(NTILES + 1):
                cur = emit_fc1(i) if i < NTILES else None
                if prev is not None:
                    emit_fc2(i - 1, prev)
                prev = cur

    nc.compile()
    return nc


_NC = None


def _get_nc():
    global _NC
    if _NC is None:
        _NC = _build()
    return _NC


def make_in_maps(data, w1, b1, w2, b2):
    data = np.asarray(data, dtype=np.float32)
    w1b = np.asarray(w1, dtype=np.float32).astype(ml_dtypes.bfloat16)
    w2b = np.asarray(w2, dtype=np.float32).astype(ml_dtypes.bfloat16)
    b1m = np.ascontiguousarray(
        np.asarray(b1, dtype=np.float32).reshape(H // 128, 128).T
    )
    b2m = np.ascontiguousarray(
        np.asarray(b2, dtype=np.float32).reshape(D // 128, 128).T
    )

    in_maps = []
    for bb in range(B):
        # residual pre-rolled by +s_t per track:
        # rolled[l, c] = data[(l - s_t) mod L, c]
        rolled = np.empty((L, D), dtype=np.float32)
        for t in range(NT):
            cs = slice(t * TS, (t + 1) * TS)
            rolled[:, cs] = np.roll(data[bb, :, cs], SEFF[t], axis=0)
        for j in range(2):
            sl = slice(j * LC, (j + 1) * LC)
            dataM = np.ascontiguousarray(
                data[bb, sl, :].T.astype(ml_dtypes.bfloat16)
            )
            dataS = np.ascontiguousarray(
                rolled[sl, :].T.astype(ml_dtypes.bfloat16)
            )
            in_maps.append({
                "dataM": dataM, "dataS": dataS,
                "w1b": w1b, "w2b": w2b, "b1m": b1m, "b2m": b2m,
            })
    return in_maps


def kernel(data, w1, b1, w2, b2):
    nc = _get_nc()
    in_maps = make_in_maps(data, w1, b1, w2, b2)
    res = bass_utils.run_bass_kernel_spmd(
        nc, in_maps, core_ids=list(range(N_CORES))
    )
    out = np.empty((B, L, D), dtype=np.float32)
    for bb in range(B):
        # accT[l, c] = out[(l - s_t) mod L, c]; undo with a -s_t roll
        accT = np.concatenate(
            [res.results[2 * bb + j]["outT"].T for j in range(2)], axis=0
        ).astype(np.float32)
        for t in range(NT):
            cs = slice(t * TS, (t + 1) * TS)
            out[bb, :, cs] = np.roll(accT[:, cs], -SEFF[t], axis=0)
    return out


# revision 25
# speedup vs baseline: 1.0279x; 1.0219x over previous
"""ChordMixerBlock Trainium2 kernel.

Math (per batch b):
    h   = gelu(data @ w1 + b1)            # exact gelu
    y   = h @ w2 + b2
    out[l, :] = rotate_chord(y)[l, :] + data[l, :]
where rotate_chord rolls track t (channels [16t, 16t+16)) forward by
s_t = 2^(t-1) positions along L (track 0: no shift; track 15: 2^14 == L
-> no shift).

Sharding: 8 cores = (batch b, L-half j); each core computes y for its own
8192-token chunk in transposed layout [256 d, 8192 l] so the contraction
dim D lands on SBUF partitions (host pre-transposes inputs and transposes
the output back).

Roll handling: entirely in data layout, zero on-device communication.
  * acc[c, p] = y[c, p] + b2[c] + dataS[c, p], where dataS is the residual
    pre-rolled by +s_t per track on the HOST (pure sharding-layout prep):
    dataS[c, p] = data[(chunk0 + p - s_t) mod L, c].  Then acc[c, p] is
    exactly out[(chunk0 + p - s_t) mod L, c] for track t's channels.
  * The device stores acc contiguously (in bf16); the HOST applies the
    per-track circular roll while unsharding (a pure index permutation of
    the gathered result, the mirror image of the input-side pre-roll).
"""

import sys

sys.path.insert(0, "/opt/trn_rl_repo")

import numpy as np
import ml_dtypes

import concourse.bass as bass
import concourse.bacc as bacc
import concourse.tile as tile
import concourse.mybir as mybir
from concourse import bass_utils

B, L, D, H = 4, 16384, 256, 512
N_CORES = 8
LC = L // 2                      # per-core chunk length
NT, TS = 16, 16                  # tracks, track size
SHIFTS = [0] + [2 ** i for i in range(NT - 1)]
SEFF = [s % L for s in SHIFTS]   # track 15 -> 0
TILE = 512                       # l-tile width for matmuls
NTILES = LC // TILE
BLK = 2048                       # DMA block width
NBLK = LC // BLK

F32 = mybir.dt.float32
BF16 = mybir.dt.bfloat16


def _build():
    nc = bacc.Bacc("TRN2", target_bir_lowering=False, debug=False, num_devices=1)

    dataM_h = nc.dram_tensor("dataM", [D, LC], BF16, kind="ExternalInput")
    dataS_h = nc.dram_tensor("dataS", [D, LC], BF16, kind="ExternalInput")
    w1_h = nc.dram_tensor("w1b", [D, H], BF16, kind="ExternalInput")
    w2_h = nc.dram_tensor("w2b", [H, D], BF16, kind="ExternalInput")
    b1_h = nc.dram_tensor("b1m", [128, H // 128], F32, kind="ExternalInput")
    b2_h = nc.dram_tensor("b2m", [128, D // 128], F32, kind="ExternalInput")
    outT_h = nc.dram_tensor("outT", [D, LC], BF16, kind="ExternalOutput")

    with tile.TileContext(nc) as tc:
        with (
            tc.tile_pool(name="const", bufs=1) as cpool,
            tc.tile_pool(name="big", bufs=1) as big,
            tc.tile_pool(name="hbf", bufs=8) as hbfp,
            tc.tile_pool(name="ph", bufs=6, space="PSUM") as php,
            tc.tile_pool(name="py", bufs=2, space="PSUM") as pyp,
        ):
            # --- persistent chunk buffers ---
            dm = [big.tile([128, LC], BF16, tag=f"dm{k}", name=f"dm{k}")
                  for k in range(2)]

            # --- weights / biases on the HWDGE rings; w1's first 128
            # columns ship as their own piece so the first LDWEIGHTS isn't
            # gated on the full tile ---
            w1sb = []
            for dt in range(2):
                w = cpool.tile([128, H], BF16, tag=f"w1_{dt}", name=f"w1sb{dt}")
                eng = nc.sync if dt == 0 else nc.scalar
                rows = slice(dt * 128, (dt + 1) * 128)
                eng.dma_start(w[:, 0:128], w1_h.ap()[rows, 0:128])
                eng.dma_start(w[:, 128:H], w1_h.ap()[rows, 128:H])
                w1sb.append(w)
            b1sb = cpool.tile([128, H // 128], F32, tag="b1")
            nc.sync.dma_start(b1sb[:], b1_h.ap())
            b2sb = cpool.tile([128, D // 128], F32, tag="b2")
            nc.scalar.dma_start(b2sb[:], b2_h.ap())
            w2sb = [cpool.tile([128, D], BF16, tag=f"w2_{ht}", name=f"w2sb{ht}")
                    for ht in range(4)]
            for ht in range(4):
                eng = nc.sync if ht % 2 == 0 else nc.scalar
                eng.dma_start(w2sb[ht][:], w2_h.ap()[ht * 128:(ht + 1) * 128, :])

            # HAM warm-up: the PE clock runs at half rate for ~10us after
            # activity starts; burn the otherwise-idle input-DMA wait on
            # dummy matmuls (zeroed scratch) so the ramp overlaps the ramp
            # of the input stream instead of the first real tiles
            warm = cpool.tile([128, TILE], BF16, tag="warm")
            nc.vector.memset(warm[:], 0.0)
            pwarm = php.tile([128, TILE], F32, tag="ph", name="ph_warm")
            for _ in range(12):
                nc.tensor.matmul(pwarm[:], warm[:, 0:128], warm[:],
                                 start=True, stop=True)

            # --- remaining persistent chunk buffers ---
            rst = [big.tile([128, LC], BF16, tag=f"rst{k}", name=f"rst{k}")
                   for k in range(2)]
            acc = [big.tile([128, LC], BF16, tag=f"acc{k}", name=f"acc{k}")
                   for k in range(2)]

            # All dm/rst input streaming goes on the SWDGE (Pool) queue:
            # each SWDGE transfer is spread across all 16 SDMA engines
            # (~1-2us for 512KB), whereas a HWDGE direct2D transfer lands on
            # ~one engine (~7-16us for 512KB).  Q7 dispatch is ~0.63us per
            # DMA and the queue never stalls (loads have no input deps), so
            # queue order == priority order.  The HWDGE rings carry only
            # weights and output stores.
            def load_dm(lo, hi):
                sl = slice(lo, hi)
                nc.gpsimd.dma_start(dm[0][:, sl], dataM_h.ap()[0:128, sl])
                nc.gpsimd.dma_start(dm[1][:, sl], dataM_h.ap()[128:256, sl])

            def load_rst(blk):
                sl = slice(blk * BLK, (blk + 1) * BLK)
                for k in range(2):
                    rows = slice(k * 128, (k + 1) * 128)
                    nc.gpsimd.dma_start(rst[k][:, sl], dataS_h.ap()[rows, sl])

            # queue order == completion priority: every dm piece the PE will
            # touch in the first few tiles goes ahead of the (later-needed)
            # rst stream, so SDMA round-robin can't starve the critical path
            load_dm(0, TILE)          # first tile's columns: smallest latency
            load_dm(TILE, 2 * TILE)
            load_dm(2 * TILE, BLK)
            load_dm(BLK, 2 * BLK)
            load_rst(0)
            load_dm(2 * BLK, 3 * BLK)
            load_rst(1)
            load_dm(3 * BLK, 4 * BLK)
            load_rst(2)
            load_rst(3)

            def emit_fc1(i):
                csl = slice(i * TILE, (i + 1) * TILE)
                hbf = []
                for ht in range(4):
                    ph = php.tile([128, TILE], F32, tag="ph",
                                  name=f"ph_{i}_{ht}")
                    hs = slice(ht * 128, (ht + 1) * 128)
                    nc.tensor.matmul(
                        ph[:], w1sb[0][:, hs], dm[0][:, csl],
                        start=True, stop=False,
                    )
                    nc.tensor.matmul(
                        ph[:], w1sb[1][:, hs], dm[1][:, csl],
                        start=False, stop=True,
                    )
                    hb = hbfp.tile([128, TILE], BF16, tag="hbf",
                                   name=f"hbf_{i}_{ht}")
                    nc.scalar.activation(
                        hb[:], ph[:], mybir.ActivationFunctionType.Gelu,
                        bias=b1sb[:, ht:ht + 1],
                    )
                    hbf.append(hb)
                return hbf

            def emit_fc2(i, hbf):
                csl = slice(i * TILE, (i + 1) * TILE)
                for k in range(2):
                    py = pyp.tile([128, TILE], F32, tag="py",
                                  name=f"py_{i}_{k}")
                    ds = slice(k * 128, (k + 1) * 128)
                    for ht in range(4):
                        nc.tensor.matmul(
                            py[:], w2sb[ht][:, ds], hbf[ht][:],
                            start=(ht == 0), stop=(ht == 3),
                        )
                    # acc = (y + b2) + pre-rolled residual, rounded to bf16
                    nc.vector.scalar_tensor_tensor(
                        acc[k][:, csl], py[:], b2sb[:, k:k + 1],
                        rst[k][:, csl],
                        mybir.AluOpType.add, mybir.AluOpType.add,
                    )
                # contiguous output store per finished 1024-col slab; the
                # final slab goes out in 512-col pieces (a HWDGE transfer
                # runs on ~one SDMA engine, so smaller pieces = lower tail
                # latency after the last matmul)
                if (i + 1) % 2 == 0:
                    if i < NTILES - 1:
                        bsl = slice((i - 1) * TILE, (i + 1) * TILE)
                        nc.sync.dma_start(outT_h.ap()[0:128, bsl], acc[0][:, bsl])
                        nc.scalar.dma_start(outT_h.ap()[128:256, bsl],
                                            acc[1][:, bsl])
                    else:
                        for lo in range((i - 1) * TILE, (i + 1) * TILE, TILE):
                            psl = slice(lo, lo + TILE)
                            nc.sync.dma_start(outT_h.ap()[0:128, psl],
                                              acc[0][:, psl])
                            nc.scalar.dma_start(outT_h.ap()[128:256, psl],
                                                acc[1][:, psl])

            # --- software-pipelined main loop: fc1(i+1) ahead of fc2(i) ---
            prev = None
            for i in range(NTILES + 1):
                cur = emit_fc1(i) if i < NTILES else None
                if prev is not None:
                    emit_fc2(i - 1, prev)
                prev = cur

    nc.compile()
    return nc


_NC = None


def _get_nc():
    global _NC
    if _NC is None:
        _NC = _build()
    return _NC


def make_in_maps(data, w1, b1, w2, b2):
    data = np.asarray(data, dtype=np.float32)
    w1b = np.asarray(w1, dtype=np.float32).astype(ml_dtypes.bfloat16)
    w2b = np.asarray(w2, dtype=np.float32).astype(ml_dtypes.bfloat16)
    b1m = np.ascontiguousarray(
        np.asarray(b1, dtype=np.float32).reshape(H // 128, 128).T
    )
    b2m = np.ascontiguousarray(
        np.asarray(b2, dtype=np.float32).reshape(D // 128, 128).T
    )

    in_maps = []
    for bb in range(B):
        # residual pre-rolled by +s_t per track:
        # rolled[l, c] = data[(l - s_t) mod L, c]
        rolled = np.empty((L, D), dtype=np.float32)
        for t in range(NT):
            cs = slice(t * TS, (t + 1) * TS)
            rolled[:, cs] = np.roll(data[bb, :, cs], SEFF[t], axis=0)
        for j in range(2):
            sl = slice(j * LC, (j + 1) * LC)
            dataM = np.ascontiguousarray(
                data[bb, sl, :].T.astype(ml_dtypes.bfloat16)
            )
            dataS = np.ascontiguousarray(
                rolled[sl, :].T.astype(ml_dtypes.bfloat16)
            )
            in_maps.append({
                "dataM": dataM, "dataS": dataS,
                "w1b": w1b, "w2b": w2b, "b1m": b1m, "b2m": b2m,
            })
    return in_maps


def kernel(data, w1, b1, w2, b2):
    nc = _get_nc()
    in_maps = make_in_maps(data, w1, b1, w2, b2)
    res = bass_utils.run_bass_kernel_spmd(
        nc, in_maps, core_ids=list(range(N_CORES))
    )
    out = np.empty((B, L, D), dtype=np.float32)
    for bb in range(B):
        # accT[l, c] = out[(l - s_t) mod L, c]; undo with a -s_t roll
        accT = np.concatenate(
            [res.results[2 * bb + j]["outT"].T for j in range(2)], axis=0
        ).astype(np.float32)
        for t in range(NT):
            cs = slice(t * TS, (t + 1) * TS)
            out[bb, :, cs] = np.roll(accT[:, cs], -SEFF[t], axis=0)
    return out


# revision 27
# speedup vs baseline: 1.0701x; 1.0411x over previous
"""ChordMixerBlock Trainium2 kernel.

Math (per batch b):
    h   = gelu(data @ w1 + b1)            # exact gelu
    y   = h @ w2 + b2
    out[l, :] = rotate_chord(y)[l, :] + data[l, :]
where rotate_chord rolls track t (channels [16t, 16t+16)) forward by
s_t = 2^(t-1) positions along L (track 0: no shift; track 15: 2^14 == L
-> no shift).

Sharding: 8 cores = (batch b, L-half j); each core computes y for its own
8192-token chunk in transposed layout [256 d, 8192 l] so the contraction
dim D lands on SBUF partitions (host pre-transposes inputs and transposes
the output back).

Roll handling: entirely in data layout, zero on-device communication.
  * acc[c, p] = y[c, p] + b2[c] + dataS[c, p], where dataS is the residual
    pre-rolled by +s_t per track on the HOST (pure sharding-layout prep):
    dataS[c, p] = data[(chunk0 + p - s_t) mod L, c].  Then acc[c, p] is
    exactly out[(chunk0 + p - s_t) mod L, c] for track t's channels.
  * The device stores acc contiguously (in bf16); the HOST applies the
    per-track circular roll while unsharding (a pure index permutation of
    the gathered result, the mirror image of the input-side pre-roll).
"""

import sys

sys.path.insert(0, "/opt/trn_rl_repo")

import numpy as np
import ml_dtypes

import concourse.bass as bass
import concourse.bacc as bacc
import concourse.tile as tile
import concourse.mybir as mybir
from concourse import bass_utils

B, L, D, H = 4, 16384, 256, 512
N_CORES = 8
LC = L // 2                      # per-core chunk length
NT, TS = 16, 16                  # tracks, track size
SHIFTS = [0] + [2 ** i for i in range(NT - 1)]
SEFF = [s % L for s in SHIFTS]   # track 15 -> 0
TILE = 512                       # l-tile width for matmuls
NTILES = LC // TILE
BLK = 2048                       # DMA block width
NBLK = LC // BLK

F32 = mybir.dt.float32
BF16 = mybir.dt.bfloat16


def _build():
    nc = bacc.Bacc("TRN2", target_bir_lowering=False, debug=False, num_devices=1)

    dataM_h = nc.dram_tensor("dataM", [D, LC], BF16, kind="ExternalInput")
    dataS_h = nc.dram_tensor("dataS", [D, LC], BF16, kind="ExternalInput")
    w1_h = nc.dram_tensor("w1b", [D, H], BF16, kind="ExternalInput")
    w2_h = nc.dram_tensor("w2b", [H, D], BF16, kind="ExternalInput")
    b1_h = nc.dram_tensor("b1m", [128, H // 128], F32, kind="ExternalInput")
    b2_h = nc.dram_tensor("b2m", [128, D // 128], F32, kind="ExternalInput")
    outT_h = nc.dram_tensor("outT", [D, LC], BF16, kind="ExternalOutput")

    with tile.TileContext(nc) as tc:
        with (
            tc.tile_pool(name="const", bufs=1) as cpool,
            tc.tile_pool(name="big", bufs=1) as big,
            tc.tile_pool(name="hbf", bufs=8) as hbfp,
            tc.tile_pool(name="ph", bufs=6, space="PSUM") as php,
            tc.tile_pool(name="py", bufs=2, space="PSUM") as pyp,
        ):
            # --- persistent chunk buffers ---
            dm = [big.tile([128, LC], BF16, tag=f"dm{k}", name=f"dm{k}")
                  for k in range(2)]

            # --- weights / biases on the HWDGE rings; w1's first 128
            # columns ship as their own piece so the first LDWEIGHTS isn't
            # gated on the full tile ---
            w1sb = []
            for dt in range(2):
                w = cpool.tile([128, H], BF16, tag=f"w1_{dt}", name=f"w1sb{dt}")
                eng = nc.sync if dt == 0 else nc.scalar
                rows = slice(dt * 128, (dt + 1) * 128)
                eng.dma_start(w[:, 0:128], w1_h.ap()[rows, 0:128])
                eng.dma_start(w[:, 128:H], w1_h.ap()[rows, 128:H])
                w1sb.append(w)
            b1sb = cpool.tile([128, H // 128], F32, tag="b1")
            nc.sync.dma_start(b1sb[:], b1_h.ap())
            b2sb = cpool.tile([128, D // 128], F32, tag="b2")
            nc.scalar.dma_start(b2sb[:], b2_h.ap())
            w2sb = [cpool.tile([128, D], BF16, tag=f"w2_{ht}", name=f"w2sb{ht}")
                    for ht in range(4)]
            for ht in range(4):
                eng = nc.sync if ht % 2 == 0 else nc.scalar
                eng.dma_start(w2sb[ht][:], w2_h.ap()[ht * 128:(ht + 1) * 128, :])

            # HAM warm-up: the PE clock runs at half rate for ~10us after
            # activity starts; burn the otherwise-idle input-DMA wait on
            # dummy matmuls (zeroed scratch) so the ramp overlaps the ramp
            # of the input stream instead of the first real tiles
            warm = cpool.tile([128, TILE], BF16, tag="warm")
            nc.vector.memset(warm[:], 0.0)
            pwarm = php.tile([128, TILE], F32, tag="ph", name="ph_warm")
            for _ in range(10):
                nc.tensor.matmul(pwarm[:], warm[:, 0:128], warm[:],
                                 start=True, stop=True)

            # --- remaining persistent chunk buffers ---
            rst = [big.tile([128, LC], BF16, tag=f"rst{k}", name=f"rst{k}")
                   for k in range(2)]
            acc = [big.tile([128, LC], BF16, tag=f"acc{k}", name=f"acc{k}")
                   for k in range(2)]

            # All dm/rst input streaming goes on the SWDGE (Pool) queue:
            # each SWDGE transfer is spread across all 16 SDMA engines
            # (~1-2us for 512KB), whereas a HWDGE direct2D transfer lands on
            # ~one engine (~7-16us for 512KB).  Q7 dispatch is ~0.63us per
            # DMA and the queue never stalls (loads have no input deps), so
            # queue order == priority order.  The HWDGE rings carry only
            # weights and output stores.
            def load_dm(lo, hi):
                sl = slice(lo, hi)
                nc.gpsimd.dma_start(dm[0][:, sl], dataM_h.ap()[0:128, sl])
                nc.gpsimd.dma_start(dm[1][:, sl], dataM_h.ap()[128:256, sl])

            def load_rst(blk):
                sl = slice(blk * BLK, (blk + 1) * BLK)
                for k in range(2):
                    rows = slice(k * 128, (k + 1) * 128)
                    nc.gpsimd.dma_start(rst[k][:, sl], dataS_h.ap()[rows, sl])

            # queue order == completion priority: every dm piece the PE will
            # touch in the first few tiles goes ahead of the (later-needed)
            # rst stream, so SDMA round-robin can't starve the critical path
            load_dm(0, TILE)          # first tile's columns: smallest latency
            load_dm(TILE, 2 * TILE)
            load_dm(2 * TILE, BLK)
            load_rst(0)               # DVE needs this ~1.5 tiles after fc1(0)
            load_dm(BLK, 2 * BLK)
            load_rst(1)
            load_dm(2 * BLK, 3 * BLK)
            load_rst(2)
            load_dm(3 * BLK, 4 * BLK)
            load_rst(3)

            def emit_fc1(i):
                csl = slice(i * TILE, (i + 1) * TILE)
                hbf = []
                for ht in range(4):
                    ph = php.tile([128, TILE], F32, tag="ph",
                                  name=f"ph_{i}_{ht}")
                    hs = slice(ht * 128, (ht + 1) * 128)
                    nc.tensor.matmul(
                        ph[:], w1sb[0][:, hs], dm[0][:, csl],
                        start=True, stop=False,
                    )
                    nc.tensor.matmul(
                        ph[:], w1sb[1][:, hs], dm[1][:, csl],
                        start=False, stop=True,
                    )
                    hb = hbfp.tile([128, TILE], BF16, tag="hbf",
                                   name=f"hbf_{i}_{ht}")
                    nc.scalar.activation(
                        hb[:], ph[:], mybir.ActivationFunctionType.Gelu,
                        bias=b1sb[:, ht:ht + 1],
                    )
                    hbf.append(hb)
                return hbf

            def emit_fc2(i, hbf):
                csl = slice(i * TILE, (i + 1) * TILE)
                for k in range(2):
                    py = pyp.tile([128, TILE], F32, tag="py",
                                  name=f"py_{i}_{k}")
                    ds = slice(k * 128, (k + 1) * 128)
                    for ht in range(4):
                        nc.tensor.matmul(
                            py[:], w2sb[ht][:, ds], hbf[ht][:],
                            start=(ht == 0), stop=(ht == 3),
                        )
                    # acc = (y + b2) + pre-rolled residual, rounded to bf16
                    nc.vector.scalar_tensor_tensor(
                        acc[k][:, csl], py[:], b2sb[:, k:k + 1],
                        rst[k][:, csl],
                        mybir.AluOpType.add, mybir.AluOpType.add,
                    )
                # contiguous output store per finished 1024-col slab; the
                # final slab goes out in 512-col pieces (a HWDGE transfer
                # runs on ~one SDMA engine, so smaller pieces = lower tail
                # latency after the last matmul)
                if (i + 1) % 2 == 0:
                    if i < NTILES - 1:
                        bsl = slice((i - 1) * TILE, (i + 1) * TILE)
                        nc.sync.dma_start(outT_h.ap()[0:128, bsl], acc[0][:, bsl])
                        nc.scalar.dma_start(outT_h.ap()[128:256, bsl],
                                            acc[1][:, bsl])
                    else:
                        for lo in range((i - 1) * TILE, (i + 1) * TILE, TILE):
                            psl = slice(lo, lo + TILE)
                            nc.sync.dma_start(outT_h.ap()[0:128, psl],
                                              acc[0][:, psl])
                            nc.scalar.dma_start(outT_h.ap()[128:256, psl],
                                                acc[1][:, psl])

            # --- software-pipelined main loop: fc1(i+1) ahead of fc2(i) ---
            prev = None
            for i in range(NTILES + 1):
                cur = emit_fc1(i) if i < NTILES else None
                if prev is not None:
                    emit_fc2(i - 1, prev)
                prev = cur

    nc.compile()
    return nc


_NC = None


def _get_nc():
    global _NC
    if _NC is None:
        _NC = _build()
    return _NC


def make_in_maps(data, w1, b1, w2, b2):
    data = np.asarray(data, dtype=np.float32)
    w1b = np.asarray(w1, dtype=np.float32).astype(ml_dtypes.bfloat16)
    w2b = np.asarray(w2, dtype=np.float32).astype(ml_dtypes.bfloat16)
    b1m = np.ascontiguousarray(
        np.asarray(b1, dtype=np.float32).reshape(H // 128, 128).T
    )
    b2m = np.ascontiguousarray(
        np.asarray(b2, dtype=np.float32).reshape(D // 128, 128).T
    )

    in_maps = []
    for bb in range(B):
        # residual pre-rolled by +s_t per track:
        # rolled[l, c] = data[(l - s_t) mod L, c]
        rolled = np.empty((L, D), dtype=np.float32)
        for t in range(NT):
            cs = slice(t * TS, (t + 1) * TS)
            rolled[:, cs] = np.roll(data[bb, :, cs], SEFF[t], axis=0)
        for j in range(2):
            sl = slice(j * LC, (j + 1) * LC)
            dataM = np.ascontiguousarray(
                data[bb, sl, :].T.astype(ml_dtypes.bfloat16)
            )
            dataS = np.ascontiguousarray(
                rolled[sl, :].T.astype(ml_dtypes.bfloat16)
            )
            in_maps.append({
                "dataM": dataM, "dataS": dataS,
                "w1b": w1b, "w2b": w2b, "b1m": b1m, "b2m": b2m,
            })
    return in_maps


def kernel(data, w1, b1, w2, b2):
    nc = _get_nc()
    in_maps = make_in_maps(data, w1, b1, w2, b2)
    res = bass_utils.run_bass_kernel_spmd(
        nc, in_maps, core_ids=list(range(N_CORES))
    )
    out = np.empty((B, L, D), dtype=np.float32)
    for bb in range(B):
        # accT[l, c] = out[(l - s_t) mod L, c]; undo with a -s_t roll
        accT = np.concatenate(
            [res.results[2 * bb + j]["outT"].T for j in range(2)], axis=0
        ).astype(np.float32)
        for t in range(NT):
            cs = slice(t * TS, (t + 1) * TS)
            out[bb, :, cs] = np.roll(accT[:, cs], -SEFF[t], axis=0)
    return out
